# revision 8
# baseline (speedup 1.0000x reference)
"""Trainium2 Bass kernel for nn_ConvBlock (SepGconv + LayerNorm + GELU MLP).

Computes, for full inputs:
    a   = einsum('bsc,brsd,dc->brc', x, kernel_basis, kernel_W) + conv_bias
    a   = LayerNorm(a) * ln_scale + ln_bias          (over channels, eps=1e-6)
    out = gelu_tanh(a @ W1 + b1) @ W2 + b2

Shapes: B=2, N=1024 (R=S=N), H=64, D=32, WF=4.

Sharding: the (B*R)=2048 output rows split into 8 contiguous shards of 256
rows, one per NeuronCore. Each core reads its kernel_basis shard once,
contracts over all S on-chip, and runs the LN/MLP tail locally. x /
weights are replicated.

Perf strategy (v5): kernel_basis is quantized to fp8 e3m4 on the host and
streamed as plain HWDGE copies on the sync queue (x first — it gates the
PE warm-up and absorbs the DMA-path ramp — then the 8 pair tiles
back-to-back) — 1 B/elem on both the HBM and SBUF side, and
the PE consumes the fp8 rhs directly (fp8 runs at bf16 speed; mixed bf16
lhsT is legal). The 256 rows form 8 pairs of 16-row j-blocks; each
pair's two matmul chains run CONCURRENTLY in the two column halves of
the 128x128 PE array (out psum partitions 0:64 / 64:128, x duplicated
into both halves of the stationary), doubling effective matmul rate so
the PE keeps pace with the DMA stream. The d-reduction multiplies PSUM
directly on DVE (no ScalarE eviction) and reduces to a bf16 aT; the last
pair's multiply/reduce is split into row halves to shorten the endgame.
The LN/MLP tail runs full-width in four 2-pair groups: channel sums for
both halves come from one matmul against a [128,2] block selector;
rsqrt(var) is a fixed-seed double-Newton chain (7 DVE ops, immediates
only): the per-core 1/v0 variance scale rides the two ACT Square scale
operands so var arrives pre-normalized near 1.0, and the seed scale
sqrt(1/v0) is folded into the broadcast selector, keeping the NEFF
core-independent. The
inv/mean*inv broadcast is one K=2 bf16 matmul and the MLP first layer
uses zero-masked W1 duplicates so each half's rows only see their own
channels. Groups are staggered two pairs behind their data; groups 0-2
run their Newton chains on the otherwise-idle GpSimd so the in-order DVE
queue never blocks the main-loop reductions, group 2 interleaves into
pair 7's matmul stream, and only group 3's chain (on DVE) trails the
final matmul, with group 2's output add emitted after it.
"""

import os

import numpy as np

import concourse.bass as bass
import concourse.tile as tile
from concourse import mybir
from concourse.bass_utils import run_bass_kernel_spmd


def _ensure_axon_hooks():
    """bass_utils imports antenv.axon_hooks when trace=True under axon; some
    images ship antenv without that module. Register a functional stand-in
    (driving NTFF capture via libaxon_pjrt.so) so tracing works, degrading
    to hook=None (no trace, run still works) if the .so is unavailable."""
    import sys
    import types

    try:
        import antenv.axon_hooks  # noqa: F401

        return
    except ImportError:
        pass
    try:
        import antenv
    except ImportError:
        antenv = types.ModuleType("antenv")
        sys.modules["antenv"] = antenv

    mod = types.ModuleType("antenv.axon_hooks")
    mod._hook = None

    def set_axon_ntff_profile_hook(h):
        mod._hook = h

    def get_axon_ntff_profile_hook():
        if mod._hook is None:
            try:
                from trn_agent_boot.trn_boot import _ntff_profile_via_ctypes

                so_path = "/opt/axon/libaxon_pjrt.so"
                if os.path.exists(so_path):
                    mod._hook = _ntff_profile_via_ctypes(so_path)
            except Exception:
                mod._hook = None
        return mod._hook

    mod.set_axon_ntff_profile_hook = set_axon_ntff_profile_hook
    mod.get_axon_ntff_profile_hook = get_axon_ntff_profile_hook
    sys.modules["antenv.axon_hooks"] = mod
    antenv.axon_hooks = mod


try:
    _ensure_axon_hooks()
except Exception:
    pass


F32 = mybir.dt.float32
BF16 = mybir.dt.bfloat16
F8 = mybir.dt.float8e3

B, N, H, D, WF = 2, 1024, 64, 32, 4
NCORES = 8
ROWS_PER_CORE = (B * N) // NCORES  # 256
RB = 16  # rows per j-block
NPAIR = 8  # pairs of j-blocks per core (each pair = 32 rows)
NK = N // 128  # 8 s-chunks of 128
FH = WF * H  # 256

# tail groups of 2 pairs (64 rows each)
GROUP_PAIRS = [[0, 1], [2, 3], [4, 5], [6, 7]]
PAIR2GRP = {}
for _g, _ps in enumerate(GROUP_PAIRS):
    for _i, _p in enumerate(_ps):
        PAIR2GRP[_p] = (_g, _i)

# packed bf16 const layout (free-dim offsets)
OFF_WB = 0          # [512]  wb: W^T broadcast over r
OFF_W1Z = 512       # [2*256] zero-masked W1 duplicates
OFF_W2 = 1024       # [128]  W2 chunks
OFF_SELS = 1152     # [2]    stats selector (1/H blocks)
BPAK_W = 1154
# packed f32 const layout
FOFF_CB = 0         # [1]  conv_bias (duplicated halves)
FOFF_B1 = 1         # [2]  b1 folded, chunked
FOFF_B2 = 3         # [64] b2 broadcast
FOFF_SC = 67        # [1]  s = sqrt(1/v0)  (per-core NR seed scale)
FOFF_CBS = 68       # [1]  conv_bias * s
FPAK_W = 69

_NC_CACHE = None
LAST_EXEC_NS = None


def _build_nc(split_waits=True):
    nc = bass.Bass(target_bir_lowering=False)

    kbh = nc.dram_tensor("kbh", [NPAIR, 128, NK, 2, RB, D], F8, kind="ExternalInput")
    xcp2 = nc.dram_tensor("xcp2", [128, NK, 128], BF16, kind="ExternalInput")
    bpak = nc.dram_tensor("bpak", [128, BPAK_W], BF16, kind="ExternalInput")
    fpak = nc.dram_tensor("fpak", [128, FPAK_W], F32, kind="ExternalInput")
    sel2 = nc.dram_tensor("sel2", [2, 128], BF16, kind="ExternalInput")
    sv2 = nc.dram_tensor("sv2", [2, 1], F32, kind="ExternalInput")
    out = nc.dram_tensor("out", [ROWS_PER_CORE, H], F32, kind="ExternalOutput")

    with tile.TileContext(nc) as tc:
        with (
            tc.tile_pool(name="consts", bufs=1) as consts,
            tc.tile_pool(name="kbp", bufs=NPAIR) as kbp,
            tc.tile_pool(name="mwp", bufs=3) as mw_pool,
            tc.tile_pool(name="work", bufs=2) as work,
            tc.tile_pool(name="pmain", bufs=3, space="PSUM") as pmain,
            tc.tile_pool(name="ptail", bufs=1, space="PSUM") as ptail,
        ):
            # ---- sync HWDGE queue: x first (gates the PE warm-up and
            # absorbs the DMA-path ramp), then the whole fp8 kernel_basis
            # stream back-to-back ----
            xc_sb = consts.tile([128, NK, 128], BF16)
            nc.sync.dma_start(out=xc_sb, in_=xcp2[:, :, :])

            kb_tiles = []
            for p in range(NPAIR):
                t = kbp.tile([128, NK, 2, RB, D], F8, name=f"kbt{p}", tag="kbt")
                if p == NPAIR - 1:
                    # split the last transfer so the final pair's matmuls
                    # start half a DMA earlier (endgame exposure)
                    half = NK // 2
                    nc.sync.dma_start(out=t[:, 0:half], in_=kbh[p, :, 0:half])
                    nc.sync.dma_start(out=t[:, half:], in_=kbh[p, :, half:])
                else:
                    nc.sync.dma_start(out=t, in_=kbh[p, :, :, :, :])
                kb_tiles.append(t)

            # ---- PE warm-up: throwaway matmuls on the x tile while the
            # first kernel_basis tiles are in flight (HAM needs ~3.4us of
            # activity to unthrottle 1.2 -> 2.4 GHz); pair 0's first cold
            # matmuls finish the warm-up window ----
            ps_warm = ptail.tile([128, 512], F32, name="ps_warm", tag="ps_warm")
            for w in range(4):
                nc.tensor.matmul(
                    ps_warm,
                    lhsT=xc_sb[:, 0, :],
                    rhs=xc_sb.rearrange("p a b -> p (a b)")[:, 0:512],
                    start=True,
                    stop=True,
                )

            # ---- packed constants on the GpSimd (SWDGE) queue ----
            bpak_t = consts.tile([128, BPAK_W], BF16)
            nc.gpsimd.dma_start(out=bpak_t, in_=bpak[:, :])
            fpak_t = consts.tile([128, FPAK_W], F32)
            nc.gpsimd.dma_start(out=fpak_t, in_=fpak[:, :])
            sel2_sb = consts.tile([2, 128], BF16)
            nc.gpsimd.dma_start(out=sel2_sb, in_=sel2[:, :])
            sv2_sb = consts.tile([2, 1], F32)
            nc.gpsimd.dma_start(out=sv2_sb, in_=sv2[:, :])

            wb_sb = bpak_t[:, OFF_WB : OFF_WB + 512]
            w1z_sb = [bpak_t[:, OFF_W1Z + FH * h : OFF_W1Z + FH * (h + 1)] for h in range(2)]
            w2_sb = bpak_t[:, OFF_W2 : OFF_W2 + 128].rearrange("p (a b) -> p a b", a=2)
            selS_sb = bpak_t[:, OFF_SELS : OFF_SELS + 2]
            cb_sb = fpak_t[:, FOFF_CB : FOFF_CB + 1]
            b1_sb = fpak_t[:, FOFF_B1 : FOFF_B1 + 2]
            b2_sb = fpak_t[:, FOFF_B2 : FOFF_B2 + 64]
            sc_sb = fpak_t[:, FOFF_SC : FOFF_SC + 1]
            cbs_sb = fpak_t[:, FOFF_CBS : FOFF_CBS + 1]

            # aT[c + 64h, 16p + r] = conv output (pre-bias) for channel c,
            # row 32p + 16h + r  (pair p in columns, j-block half h in the
            # partition halves)
            aT = consts.tile([128, NPAIR * RB], BF16)

            state = {}

            # ---------------- tail ----------------
            def t_stats(g):
                st = state[("st", g)]
                # ps_s[h, a, (pr, rl)] = [mean | mean-of-squares] of half h
                # (selS holds 1/H in the two diagonal blocks)
                ps_s = ptail.tile([2, 2, 2 * RB], F32, name=f"ps_s{g}", tag="ps_s")
                nc.tensor.matmul(
                    ps_s,
                    lhsT=selS_sb,
                    rhs=st.rearrange("p a q r -> p a (q r)"),
                    start=True,
                    stop=True,
                )
                # mean^2 on ScalarE (Square is a filler in every act table)
                msq = work.tile([2, 2 * RB], F32, name=f"msq{g}", tag="msq")
                nc.scalar.activation(
                    out=msq,
                    in_=ps_s[:, 0, :],
                    func=mybir.ActivationFunctionType.Square,
                    bias=0.0,
                    scale=sv2_sb,
                )
                state[("ps_s", g)] = ps_s
                state[("msq", g)] = msq

            def t_math(g, gp=False):
                # var' = (E[x^2] - mean^2)/v0 lands near 1 (the 1/v0 scale
                # rides the two ACT Square scale operands), so rsqrt is a
                # double Newton-Raphson from the constant seed 1.0 with
                # NEFF-constant immediates; the seed scale sqrt(1/v0) is
                # restored by the sel2 broadcast. rp[:,1] = mean * rp[:,0].
                # gp=True runs the chain on the otherwise-idle GpSimd (via a
                # ScalarE PSUM evict — GpSimd has no PSUM port) so it never
                # blocks the main-loop reductions in the in-order DVE queue.
                Q = 2 * RB
                ps_s = state[("ps_s", g)]
                if gp:
                    eng = nc.gpsimd
                    msb = work.tile([2, 2, Q], F32, name=f"msb{g}", tag="msb")
                    nc.scalar.activation(
                        out=msb, in_=ps_s,
                        func=mybir.ActivationFunctionType.Copy,
                        bias=0.0, scale=1.0,
                    )
                    ps_s = msb
                else:
                    eng = nc.vector
                var = work.tile([2, Q], F32, name=f"var{g}", tag="var")
                eng.tensor_sub(var, ps_s[:, 1, :], state[("msq", g)])
                u1 = work.tile([2, Q], F32, name=f"u1_{g}", tag="u1")
                eng.tensor_scalar(
                    out=u1, in0=var, scalar1=-0.5, scalar2=1.5,
                    op0=mybir.AluOpType.mult, op1=mybir.AluOpType.add,
                )
                t1 = work.tile([2, Q], F32, name=f"t1_{g}", tag="t1")
                eng.tensor_mul(t1, var, u1)
                eng.tensor_mul(t1, t1, u1)  # var*u1^2
                q1 = work.tile([2, Q], F32, name=f"q1_{g}", tag="q1")
                eng.tensor_scalar(
                    out=q1, in0=t1, scalar1=-0.5, scalar2=1.5,
                    op0=mybir.AluOpType.mult, op1=mybir.AluOpType.add,
                )
                rp = work.tile([2, 2, Q], BF16, name=f"rp{g}", tag="rp")
                eng.tensor_mul(rp[:, 0, :], u1, q1)
                eng.tensor_mul(rp[:, 1, :], ps_s[:, 0, :], rp[:, 0, :])
                state[("rp", g)] = rp

            def t_bc_aln(g):
                rp = state[("rp", g)]
                st = state[("st", g)]
                # one K=2 matmul broadcasts each half's [inv | mu*inv] row
                # to that half's 64 channel partitions (sel2 = sqrt(c) in
                # the indicator blocks restores the seed scale)
                ps_bc = ptail.tile([128, 2, 2 * RB], F32, name=f"ps_bc{g}", tag="ps_bc")
                nc.tensor.matmul(ps_bc, lhsT=sel2_sb, rhs=rp, start=True, stop=True)
                # ln_scale/ln_bias are folded into W1/b1 on the host, so the
                # normalized activation is just st*inv - mu*inv
                aln = work.tile([128, 2 * RB], BF16, name=f"aln{g}", tag="aln")
                nc.vector.tensor_mul(
                    aln, st.rearrange("p a q r -> p a (q r)")[:, 0, :], ps_bc[:, 0, :]
                )
                nc.vector.tensor_sub(aln, aln, ps_bc[:, 1, :])
                state[("aln", g)] = aln

            def t_mlp_mm(g):
                aln = state[("aln", g)]
                # h^T: each half h of the rows contracts only its own channel
                # partitions via the zero-masked W1 duplicate w1z[h];
                # fh-outer order so gelu(fh=0) can start after two matmuls
                ph = ptail.tile(
                    [128, 2, 2, 2, RB], F32, name=f"ph{g}", tag="ph"
                )
                for fh in range(2):
                    for h in range(2):
                        nc.tensor.matmul(
                            ph[:, fh, :, h, :],
                            lhsT=w1z_sb[h][:, 128 * fh : 128 * (fh + 1)],
                            rhs=aln.rearrange("p (q r) -> p q r", q=2),
                            start=True,
                            stop=True,
                        )
                hT = work.tile([128, 2, 4 * RB], BF16, name=f"hT{g}", tag="hT")
                for fh in range(2):
                    nc.scalar.activation(
                        out=hT[:, fh, :],
                        in_=ph[:, fh, :, :, :],
                        func=mybir.ActivationFunctionType.Gelu_apprx_tanh,
                        bias=b1_sb[:, fh : fh + 1],
                        scale=1.0,
                    )
                po = ptail.tile([64, H], F32, name=f"po{g}", tag="po")
                for fh in range(2):
                    nc.tensor.matmul(
                        po,
                        lhsT=hT[:, fh, :],
                        rhs=w2_sb[:, fh, :],
                        start=(fh == 0),
                        stop=(fh == 1),
                    )
                state[("po", g)] = po

            def t_out(g):
                po = state[("po", g)]
                o_sb = work.tile([64, H], F32, name=f"o_sb{g}", tag="o_sb")
                nc.vector.tensor_add(o_sb, po, b2_sb[0:64, :])
                nc.sync.dma_start(out=out[64 * g : 64 * (g + 1), :], in_=o_sb)

            def t_mlp(g):
                t_mlp_mm(g)
                t_out(g)

            # per-pair piece of the group's stacked [a+cb | (a+cb)^2]
            def t_stacked(p):
                g, pr = PAIR2GRP[p]
                if pr == 0:
                    state[("st", g)] = work.tile(
                        [128, 2, 2, RB], BF16, name=f"st{g}", tag="st"
                    )
                st = state[("st", g)]
                sl = slice(RB * p, RB * (p + 1))
                nc.vector.tensor_scalar(
                    out=st[:, 0, pr, :], in0=aT[:, sl],
                    scalar1=cb_sb, scalar2=None, op0=mybir.AluOpType.add,
                )
                nc.scalar.activation(
                    out=st[:, 1, pr, :], in_=aT[:, sl],
                    func=mybir.ActivationFunctionType.Square,
                    bias=cbs_sb, scale=sc_sb,
                )

            # tail work staggered through the pair loop, two pairs behind
            # its data; group 2 interleaves into pair 7's k-loop and group
            # 3 (pairs 6,7) is the endgame
            sched = {
                3: [lambda: t_stats(0), lambda: t_math(0, gp=True)],
                4: [lambda: t_bc_aln(0), lambda: t_mlp(0)],
                5: [lambda: t_stats(1), lambda: t_math(1, gp=True)],
                6: [lambda: t_bc_aln(1), lambda: t_mlp(1)],
            }
            k_sched_p7 = {
                1: [lambda: t_stats(2), lambda: t_math(2, gp=True)],
                6: [lambda: t_bc_aln(2)],
            }

            # ---------------- main contraction ----------------
            for p in range(NPAIR):
                kbt = kb_tiles[p]
                ps = pmain.tile([128, RB, D], F32, name="ps", tag="ps")
                for k in range(NK):
                    # the pair's two j-blocks run concurrently in the two
                    # column halves of the PE array (col groups from the
                    # psum base partition; x duplicated into both halves)
                    nc.tensor.matmul(
                        ps[0:64],
                        lhsT=xc_sb[:, k, 0:64],
                        rhs=kbt[:, k, 0, :, :],
                        start=(k == 0),
                        stop=(k == NK - 1),
                    )
                    nc.tensor.matmul(
                        ps[64:128],
                        lhsT=xc_sb[:, k, 64:128],
                        rhs=kbt[:, k, 1, :, :],
                        start=(k == 0),
                        stop=(k == NK - 1),
                    )
                    if p == NPAIR - 1:
                        for fn in k_sched_p7.get(k, ()):
                            fn()
                # d-reduction: DVE multiplies PSUM directly by the W^T
                # broadcast and reduces over d into the bf16 aT; the last
                # pair is split into row halves to shorten the endgame
                halves = (
                    [(0, RB // 2), (RB // 2, RB)] if p == NPAIR - 1 else [(0, RB)]
                )
                for r0, r1 in halves:
                    nr = r1 - r0
                    mw = mw_pool.tile([128, nr, D], BF16, name=f"mw{nr}", tag=f"mw{nr}")
                    nc.vector.tensor_mul(
                        mw.rearrange("p a b -> p (a b)"),
                        ps[:, r0:r1, :].rearrange("p a b -> p (a b)"),
                        wb_sb[:, r0 * D : r1 * D],
                    )
                    with nc.allow_low_precision(
                        reason="bf16 aT validated: fro rel err 1.55e-2 vs 2e-2 gate"
                    ):
                        nc.vector.tensor_reduce(
                            out=aT[:, RB * p + r0 : RB * p + r1],
                            in_=mw,
                            axis=mybir.AxisListType.X,
                            op=mybir.AluOpType.add,
                        )
                t_stacked(p)
                for fn in sched.get(p, ()):
                    fn()

            # endgame: group 2's matmuls overlap the pair-7 reduction;
            # its output add is emitted after group 3's DVE chain so the
            # in-order DVE queue never stalls on it
            t_mlp_mm(2)
            t_stats(3)
            t_math(3)
            t_out(2)
            t_bc_aln(3)
            t_mlp_mm(3)
            t_out(3)

    if split_waits:
        _split_matmul_waits(nc)
    return nc


def _split_matmul_waits(nc):
    """This walrus build rejects engine instructions carrying more than one
    semaphore wait ("Too many sync wait commands"). Peel all but the last
    wait off onto same-engine NoOps inserted immediately before the
    instruction — NoOps execute in queue order on the same sequencer, so the
    wait semantics are unchanged."""
    f = nc.m.functions[0]
    nop_id = 0
    for blk in f.blocks:
        insts = list(blk.instructions)
        out = []
        changed = False
        for inst in insts:
            si = inst.sync_info
            if (
                si is not None
                and si.on_wait is not None
                and len(si.on_wait) > 1
                and getattr(inst, "engine", None) is not None
            ):
                waits = list(si.on_wait)
                for w in waits[:-1]:
                    nop = mybir.InstNoOp(
                        name=f"I-mmwait-{nop_id}",
                        engine=inst.engine,
                        ins=[],
                        outs=[],
                        sync_info=mybir.SyncInfo(on_wait=[w], on_update=[]),
                    )
                    nop_id += 1
                    out.append(nop)
                inst.sync_info = mybir.SyncInfo(
                    on_wait=[waits[-1]], on_update=list(si.on_update or [])
                )
                changed = True
            out.append(inst)
        if changed:
            blk.instructions = out


def _get_nc():
    global _NC_CACHE
    if _NC_CACHE is None:
        _NC_CACHE = _build_nc()
    return _NC_CACHE


def _prep_shared(kernel_W, conv_bias, ln_scale, ln_bias, W1, b1, W2, b2):
    import ml_dtypes

    WT = kernel_W.T.astype(np.float32)  # [H, D]
    wb = np.broadcast_to(WT[:, None, :], (H, RB, D)).reshape(H, RB * D)
    # fold LayerNorm affine into the first MLP layer: the kernel computes
    # z = (a - mu) * inv_std, and  (z*s + b) @ W1 + b1 = z @ (s[:,None]*W1)
    # + (b1 + b @ W1)
    W1f = ln_scale[:, None].astype(np.float32) * W1.astype(np.float32)
    b1f = b1.astype(np.float32) + ln_bias.astype(np.float32) @ W1.astype(np.float32)

    bpak = np.zeros((128, BPAK_W), np.float32)
    bpak[0:64, OFF_WB : OFF_WB + 512] = wb
    bpak[64:128, OFF_WB : OFF_WB + 512] = wb
    bpak[0:64, OFF_W1Z : OFF_W1Z + FH] = W1f
    bpak[64:128, OFF_W1Z + FH : OFF_W1Z + 2 * FH] = W1f
    bpak[:, OFF_W2 : OFF_W2 + 128] = (
        W2.reshape(2, 128, H).transpose(1, 0, 2).reshape(128, 128)
    )
    bpak[0:64, OFF_SELS] = 1.0 / H
    bpak[64:128, OFF_SELS + 1] = 1.0 / H

    fpak = np.zeros((128, FPAK_W), np.float32)
    fpak[:, FOFF_CB] = np.tile(conv_bias, 2)
    fpak[:, FOFF_B1 : FOFF_B1 + 2] = b1f.reshape(2, 128).T
    fpak[0:64, FOFF_B2 : FOFF_B2 + 64] = np.broadcast_to(b2, (64, H))

    return dict(bpak=np.ascontiguousarray(bpak.astype(ml_dtypes.bfloat16))), fpak


def _prep_core_scale(xb, kernel_W, fpak, conv_bias):
    """Per-core NR constants: v0 = typical LN row variance estimate. The
    seed scale s = bf16(1/sqrt(v0)) rides the two ACT Square scales (so
    var arrives pre-divided by v0 and the NR runs from seed 1.0 with
    NEFF-constant immediates) and is restored via the sel2 broadcast."""
    import ml_dtypes

    WT = kernel_W.T.astype(np.float32)
    v0 = float(np.mean((xb.astype(np.float32) ** 2).sum(0) * (WT**2).sum(1)))
    s = np.float32(
        np.float32(1.0 / np.sqrt(v0)).astype(ml_dtypes.bfloat16).astype(np.float32)
    )
    sel2 = np.zeros((2, 128), np.float32)
    sel2[0, 0:64] = s
    sel2[1, 64:128] = s
    sv2 = np.full((2, 1), s, np.float32)
    fpc = fpak.copy()
    fpc[:, FOFF_SC] = s
    fpc[:, FOFF_CBS] = np.tile(conv_bias, 2) * s
    return (
        np.ascontiguousarray(sel2.astype(ml_dtypes.bfloat16)),
        np.ascontiguousarray(sv2),
        np.ascontiguousarray(fpc),
    )


def _prep_x(xb):
    import ml_dtypes

    # (N, H) -> (128, k, 2*H) bf16, with s = 128*k + p and x duplicated into
    # both column halves of the stationary operand
    xh = xb.astype(ml_dtypes.bfloat16)
    base = xh.reshape(NK, 128, H).transpose(1, 0, 2)
    return np.ascontiguousarray(np.concatenate([base, base], axis=2))


def _prep_kb_shard(shard):
    import ml_dtypes

    # shard (256, 1024, 32) f32 -> [pair, s%128, k, half, r, d] fp8 e3m4
    q = shard.reshape(NPAIR, 2, RB, NK, 128, D).transpose(0, 4, 3, 1, 2, 5)
    return np.ascontiguousarray(q.astype(ml_dtypes.float8_e3m4))


def kernel(
    x,
    kernel_basis,
    kernel_W,
    conv_bias,
    ln_scale,
    ln_bias,
    W1,
    b1,
    W2,
    b2,
):
    global LAST_EXEC_NS
    x = np.ascontiguousarray(np.asarray(x, np.float32))
    kb = np.ascontiguousarray(np.asarray(kernel_basis, np.float32))
    kernel_W = np.asarray(kernel_W, np.float32)
    conv_bias = np.asarray(conv_bias, np.float32)
    shared, fpak0 = _prep_shared(
        kernel_W,
        conv_bias,
        np.asarray(ln_scale, np.float32),
        np.asarray(ln_bias, np.float32),
        np.asarray(W1, np.float32),
        np.asarray(b1, np.float32),
        np.asarray(W2, np.float32),
        np.asarray(b2, np.float32),
    )
    xps = [_prep_x(x[b]) for b in range(B)]
    scs = [_prep_core_scale(x[b], kernel_W, fpak0, conv_bias) for b in range(B)]

    kbf = kb.reshape(B * N, N, D)
    in_maps = []
    for c in range(NCORES):
        b = c // (NCORES // B)
        hi = _prep_kb_shard(kbf[c * ROWS_PER_CORE : (c + 1) * ROWS_PER_CORE])
        in_maps.append(
            dict(
                kbh=hi, xcp2=xps[b], sel2=scs[b][0], sv2=scs[b][1],
                fpak=scs[b][2], **shared,
            )
        )

    nc = _get_nc()
    trace = bool(os.environ.get("KERNEL_BASS_TRACE"))
    res = run_bass_kernel_spmd(nc, in_maps, core_ids=list(range(NCORES)), trace=trace)
    LAST_EXEC_NS = res.exec_time_ns

    outs = np.concatenate([res.results[c]["out"] for c in range(NCORES)], axis=0)
    return outs.reshape(B, N, H)


# revision 9
# speedup vs baseline: 1.0421x; 1.0421x over previous
"""Trainium2 Bass kernel for nn_ConvBlock (SepGconv + LayerNorm + GELU MLP).

Computes, for full inputs:
    a   = einsum('bsc,brsd,dc->brc', x, kernel_basis, kernel_W) + conv_bias
    a   = LayerNorm(a) * ln_scale + ln_bias          (over channels, eps=1e-6)
    out = gelu_tanh(a @ W1 + b1) @ W2 + b2

Shapes: B=2, N=1024 (R=S=N), H=64, D=32, WF=4.

Sharding: the (B*R)=2048 output rows split into 8 contiguous shards of 256
rows, one per NeuronCore. Each core reads its kernel_basis shard once,
contracts over all S on-chip, and runs the LN/MLP tail locally. x /
weights are replicated.

Perf strategy (v5): kernel_basis is quantized to fp8 e3m4 on the host and
streamed as plain HWDGE copies on the sync queue (x first — it gates the
PE warm-up and absorbs the DMA-path ramp — then the 8 pair tiles
back-to-back) — 1 B/elem on both the HBM and SBUF side, and
the PE consumes the fp8 rhs directly (fp8 runs at bf16 speed; mixed bf16
lhsT is legal). The 256 rows form 8 pairs of 16-row j-blocks; each
pair's two matmul chains run CONCURRENTLY in the two column halves of
the 128x128 PE array (out psum partitions 0:64 / 64:128, x duplicated
into both halves of the stationary), doubling effective matmul rate so
the PE keeps pace with the DMA stream. The d-reduction multiplies PSUM
directly on DVE (no ScalarE eviction) and reduces to a bf16 aT; the last
pair's multiply/reduce is split into row halves to shorten the endgame.
The LN/MLP tail runs full-width in four 2-pair groups: channel sums for
both halves come from one matmul against a [128,2] block selector;
rsqrt(var) is a fixed-seed double-Newton chain (7 DVE ops, immediates
only): the per-core 1/v0 variance scale rides the two ACT Square scale
operands so var arrives pre-normalized near 1.0, and the seed scale
sqrt(1/v0) is folded into the broadcast selector, keeping the NEFF
core-independent. The
inv/mean*inv broadcast is one K=2 bf16 matmul and the MLP first layer
uses zero-masked W1 duplicates so each half's rows only see their own
channels. Groups are staggered two pairs behind their data; groups 0-2
run their Newton chains on the otherwise-idle GpSimd so the in-order DVE
queue never blocks the main-loop reductions, group 2 interleaves into
pair 7's matmul stream, and only group 3's chain (on DVE) trails the
final matmul, with group 2's output add emitted after it.
"""

import os

import numpy as np

import concourse.bass as bass
import concourse.tile as tile
from concourse import mybir
from concourse.bass_utils import run_bass_kernel_spmd


def _ensure_axon_hooks():
    """bass_utils imports antenv.axon_hooks when trace=True under axon; some
    images ship antenv without that module. Register a functional stand-in
    (driving NTFF capture via libaxon_pjrt.so) so tracing works, degrading
    to hook=None (no trace, run still works) if the .so is unavailable."""
    import sys
    import types

    try:
        import antenv.axon_hooks  # noqa: F401

        return
    except ImportError:
        pass
    try:
        import antenv
    except ImportError:
        antenv = types.ModuleType("antenv")
        sys.modules["antenv"] = antenv

    mod = types.ModuleType("antenv.axon_hooks")
    mod._hook = None

    def set_axon_ntff_profile_hook(h):
        mod._hook = h

    def get_axon_ntff_profile_hook():
        if mod._hook is None:
            try:
                from trn_agent_boot.trn_boot import _ntff_profile_via_ctypes

                so_path = "/opt/axon/libaxon_pjrt.so"
                if os.path.exists(so_path):
                    mod._hook = _ntff_profile_via_ctypes(so_path)
            except Exception:
                mod._hook = None
        return mod._hook

    mod.set_axon_ntff_profile_hook = set_axon_ntff_profile_hook
    mod.get_axon_ntff_profile_hook = get_axon_ntff_profile_hook
    sys.modules["antenv.axon_hooks"] = mod
    antenv.axon_hooks = mod


try:
    _ensure_axon_hooks()
except Exception:
    pass


F32 = mybir.dt.float32
BF16 = mybir.dt.bfloat16
F8 = mybir.dt.float8e3

B, N, H, D, WF = 2, 1024, 64, 32, 4
NCORES = 8
ROWS_PER_CORE = (B * N) // NCORES  # 256
RB = 16  # rows per j-block
NPAIR = 8  # pairs of j-blocks per core (each pair = 32 rows)
NK = N // 128  # 8 s-chunks of 128
FH = WF * H  # 256

# tail groups of 2 pairs (64 rows each)
GROUP_PAIRS = [[0, 1], [2, 3], [4, 5], [6, 7]]
PAIR2GRP = {}
for _g, _ps in enumerate(GROUP_PAIRS):
    for _i, _p in enumerate(_ps):
        PAIR2GRP[_p] = (_g, _i)

# packed bf16 const layout (free-dim offsets)
OFF_WB = 0          # [512]  wb: W^T broadcast over r
OFF_W1Z = 512       # [2*256] zero-masked W1 duplicates
OFF_W2 = 1024       # [128]  W2 chunks
OFF_SELS = 1152     # [2]    stats selector (1/H blocks)
BPAK_W = 1154
# packed f32 const layout
FOFF_CB = 0         # [1]  conv_bias (duplicated halves)
FOFF_B1 = 1         # [2]  b1 folded, chunked
FOFF_B2 = 3         # [64] b2 broadcast
FOFF_SC = 67        # [1]  s = sqrt(1/v0)  (per-core NR seed scale)
FOFF_CBS = 68       # [1]  conv_bias * s
FPAK_W = 69

_NC_CACHE = None
LAST_EXEC_NS = None


def _build_nc(split_waits=True):
    nc = bass.Bass(target_bir_lowering=False)

    kbh = nc.dram_tensor("kbh", [NPAIR, 128, NK, 2, RB, D], F8, kind="ExternalInput")
    xcp2 = nc.dram_tensor("xcp2", [128, NK, 128], BF16, kind="ExternalInput")
    bpak = nc.dram_tensor("bpak", [128, BPAK_W], BF16, kind="ExternalInput")
    fpak = nc.dram_tensor("fpak", [128, FPAK_W], F32, kind="ExternalInput")
    sel2 = nc.dram_tensor("sel2", [2, 128], BF16, kind="ExternalInput")
    sv2 = nc.dram_tensor("sv2", [2, 1], F32, kind="ExternalInput")
    out = nc.dram_tensor("out", [ROWS_PER_CORE, H], F32, kind="ExternalOutput")

    with tile.TileContext(nc) as tc:
        with (
            tc.tile_pool(name="consts", bufs=1) as consts,
            tc.tile_pool(name="kbp", bufs=NPAIR) as kbp,
            tc.tile_pool(name="mwp", bufs=3) as mw_pool,
            tc.tile_pool(name="work", bufs=2) as work,
            tc.tile_pool(name="pmain", bufs=3, space="PSUM") as pmain,
            tc.tile_pool(name="ptail", bufs=1, space="PSUM") as ptail,
        ):
            # ---- sync HWDGE queue: x first (gates the PE warm-up and
            # absorbs the DMA-path ramp), then the whole fp8 kernel_basis
            # stream back-to-back ----
            xc_sb = consts.tile([128, NK, 128], BF16)
            nc.sync.dma_start(out=xc_sb, in_=xcp2[:, :, :])

            kb_tiles = []
            for p in range(NPAIR):
                t = kbp.tile([128, NK, 2, RB, D], F8, name=f"kbt{p}", tag="kbt")
                if p == NPAIR - 1:
                    # split the last transfer so the final pair's matmuls
                    # start half a DMA earlier (endgame exposure)
                    half = NK // 2
                    nc.sync.dma_start(out=t[:, 0:half], in_=kbh[p, :, 0:half])
                    nc.sync.dma_start(out=t[:, half:], in_=kbh[p, :, half:])
                else:
                    nc.sync.dma_start(out=t, in_=kbh[p, :, :, :, :])
                kb_tiles.append(t)

            # ---- PE warm-up: throwaway matmuls on the x tile while the
            # first kernel_basis tiles are in flight (HAM needs ~3.4us of
            # activity to unthrottle 1.2 -> 2.4 GHz); pair 0's first cold
            # matmuls finish the warm-up window ----
            ps_warm = ptail.tile([128, 512], F32, name="ps_warm", tag="ps_warm")
            for w in range(6):
                nc.tensor.matmul(
                    ps_warm,
                    lhsT=xc_sb[:, 0, :],
                    rhs=xc_sb.rearrange("p a b -> p (a b)")[:, 0:512],
                    start=True,
                    stop=True,
                )

            # ---- packed constants on the GpSimd (SWDGE) queue ----
            bpak_t = consts.tile([128, BPAK_W], BF16)
            nc.gpsimd.dma_start(out=bpak_t, in_=bpak[:, :])
            fpak_t = consts.tile([128, FPAK_W], F32)
            nc.gpsimd.dma_start(out=fpak_t, in_=fpak[:, :])
            sel2_sb = consts.tile([2, 128], BF16)
            nc.gpsimd.dma_start(out=sel2_sb, in_=sel2[:, :])
            sv2_sb = consts.tile([2, 1], F32)
            nc.gpsimd.dma_start(out=sv2_sb, in_=sv2[:, :])

            wb_sb = bpak_t[:, OFF_WB : OFF_WB + 512]
            w1z_sb = [bpak_t[:, OFF_W1Z + FH * h : OFF_W1Z + FH * (h + 1)] for h in range(2)]
            w2_sb = bpak_t[:, OFF_W2 : OFF_W2 + 128].rearrange("p (a b) -> p a b", a=2)
            selS_sb = bpak_t[:, OFF_SELS : OFF_SELS + 2]
            cb_sb = fpak_t[:, FOFF_CB : FOFF_CB + 1]
            b1_sb = fpak_t[:, FOFF_B1 : FOFF_B1 + 2]
            b2_sb = fpak_t[:, FOFF_B2 : FOFF_B2 + 64]
            sc_sb = fpak_t[:, FOFF_SC : FOFF_SC + 1]
            cbs_sb = fpak_t[:, FOFF_CBS : FOFF_CBS + 1]

            # aT[c + 64h, 16p + r] = conv output (pre-bias) for channel c,
            # row 32p + 16h + r  (pair p in columns, j-block half h in the
            # partition halves)
            aT = consts.tile([128, NPAIR * RB], BF16)

            state = {}

            # ---------------- tail ----------------
            def t_stats(g):
                st = state[("st", g)]
                # ps_s[h, a, (pr, rl)] = [mean | mean-of-squares] of half h
                # (selS holds 1/H in the two diagonal blocks)
                ps_s = ptail.tile([2, 2, 2 * RB], F32, name=f"ps_s{g}", tag="ps_s")
                nc.tensor.matmul(
                    ps_s,
                    lhsT=selS_sb,
                    rhs=st.rearrange("p a q r -> p a (q r)"),
                    start=True,
                    stop=True,
                )
                # mean^2 on ScalarE (Square is a filler in every act table)
                msq = work.tile([2, 2 * RB], F32, name=f"msq{g}", tag="msq")
                nc.scalar.activation(
                    out=msq,
                    in_=ps_s[:, 0, :],
                    func=mybir.ActivationFunctionType.Square,
                    bias=0.0,
                    scale=sv2_sb,
                )
                state[("ps_s", g)] = ps_s
                state[("msq", g)] = msq

            def t_math(g, gp=False):
                # var' = (E[x^2] - mean^2)/v0 lands near 1 (the 1/v0 scale
                # rides the two ACT Square scale operands), so rsqrt is a
                # double Newton-Raphson from the constant seed 1.0 with
                # NEFF-constant immediates; the seed scale sqrt(1/v0) is
                # restored by the sel2 broadcast. rp[:,1] = mean * rp[:,0].
                # gp=True runs the chain on the otherwise-idle GpSimd (via a
                # ScalarE PSUM evict — GpSimd has no PSUM port) so it never
                # blocks the main-loop reductions in the in-order DVE queue.
                Q = 2 * RB
                ps_s = state[("ps_s", g)]
                if gp:
                    eng = nc.gpsimd
                    msb = work.tile([2, 2, Q], F32, name=f"msb{g}", tag="msb")
                    nc.scalar.activation(
                        out=msb, in_=ps_s,
                        func=mybir.ActivationFunctionType.Copy,
                        bias=0.0, scale=1.0,
                    )
                    ps_s = msb
                else:
                    eng = nc.vector
                var = work.tile([2, Q], F32, name=f"var{g}", tag="var")
                eng.tensor_sub(var, ps_s[:, 1, :], state[("msq", g)])
                u1 = work.tile([2, Q], F32, name=f"u1_{g}", tag="u1")
                eng.tensor_scalar(
                    out=u1, in0=var, scalar1=-0.5, scalar2=1.5,
                    op0=mybir.AluOpType.mult, op1=mybir.AluOpType.add,
                )
                t1 = work.tile([2, Q], F32, name=f"t1_{g}", tag="t1")
                eng.tensor_mul(t1, var, u1)
                eng.tensor_mul(t1, t1, u1)  # var*u1^2
                q1 = work.tile([2, Q], F32, name=f"q1_{g}", tag="q1")
                eng.tensor_scalar(
                    out=q1, in0=t1, scalar1=-0.5, scalar2=1.5,
                    op0=mybir.AluOpType.mult, op1=mybir.AluOpType.add,
                )
                rp = work.tile([2, 2, Q], BF16, name=f"rp{g}", tag="rp")
                eng.tensor_mul(rp[:, 0, :], u1, q1)
                eng.tensor_mul(rp[:, 1, :], ps_s[:, 0, :], rp[:, 0, :])
                state[("rp", g)] = rp

            def t_bc_aln(g):
                rp = state[("rp", g)]
                st = state[("st", g)]
                # one K=2 matmul broadcasts each half's [inv | mu*inv] row
                # to that half's 64 channel partitions (sel2 = sqrt(c) in
                # the indicator blocks restores the seed scale)
                ps_bc = ptail.tile([128, 2, 2 * RB], F32, name=f"ps_bc{g}", tag="ps_bc")
                nc.tensor.matmul(ps_bc, lhsT=sel2_sb, rhs=rp, start=True, stop=True)
                # ln_scale/ln_bias are folded into W1/b1 on the host, so the
                # normalized activation is just st*inv - mu*inv
                aln = work.tile([128, 2 * RB], BF16, name=f"aln{g}", tag="aln")
                nc.vector.tensor_mul(
                    aln, st.rearrange("p a q r -> p a (q r)")[:, 0, :], ps_bc[:, 0, :]
                )
                nc.vector.tensor_sub(aln, aln, ps_bc[:, 1, :])
                state[("aln", g)] = aln

            def t_mlp_mm(g):
                aln = state[("aln", g)]
                # h^T: each half h of the rows contracts only its own channel
                # partitions via the zero-masked W1 duplicate w1z[h];
                # fh-outer order so gelu(fh=0) can start after two matmuls
                ph = ptail.tile(
                    [128, 2, 2, 2, RB], F32, name=f"ph{g}", tag="ph"
                )
                for fh in range(2):
                    for h in range(2):
                        nc.tensor.matmul(
                            ph[:, fh, :, h, :],
                            lhsT=w1z_sb[h][:, 128 * fh : 128 * (fh + 1)],
                            rhs=aln.rearrange("p (q r) -> p q r", q=2),
                            start=True,
                            stop=True,
                        )
                hT = work.tile([128, 2, 4 * RB], BF16, name=f"hT{g}", tag="hT")
                for fh in range(2):
                    nc.scalar.activation(
                        out=hT[:, fh, :],
                        in_=ph[:, fh, :, :, :],
                        func=mybir.ActivationFunctionType.Gelu_apprx_tanh,
                        bias=b1_sb[:, fh : fh + 1],
                        scale=1.0,
                    )
                po = ptail.tile([64, H], F32, name=f"po{g}", tag="po")
                for fh in range(2):
                    nc.tensor.matmul(
                        po,
                        lhsT=hT[:, fh, :],
                        rhs=w2_sb[:, fh, :],
                        start=(fh == 0),
                        stop=(fh == 1),
                    )
                state[("po", g)] = po

            def t_out(g):
                po = state[("po", g)]
                o_sb = work.tile([64, H], F32, name=f"o_sb{g}", tag="o_sb")
                nc.vector.tensor_add(o_sb, po, b2_sb[0:64, :])
                nc.sync.dma_start(out=out[64 * g : 64 * (g + 1), :], in_=o_sb)

            def t_mlp(g):
                t_mlp_mm(g)
                t_out(g)

            # per-pair piece of the group's stacked [a+cb | (a+cb)^2]
            def t_stacked(p):
                g, pr = PAIR2GRP[p]
                if pr == 0:
                    state[("st", g)] = work.tile(
                        [128, 2, 2, RB], BF16, name=f"st{g}", tag="st"
                    )
                st = state[("st", g)]
                sl = slice(RB * p, RB * (p + 1))
                nc.vector.tensor_scalar(
                    out=st[:, 0, pr, :], in0=aT[:, sl],
                    scalar1=cb_sb, scalar2=None, op0=mybir.AluOpType.add,
                )
                nc.scalar.activation(
                    out=st[:, 1, pr, :], in_=aT[:, sl],
                    func=mybir.ActivationFunctionType.Square,
                    bias=cbs_sb, scale=sc_sb,
                )

            # tail work staggered through the pair loop, two pairs behind
            # its data; group 2 interleaves into pair 7's k-loop and group
            # 3 (pairs 6,7) is the endgame
            sched = {
                3: [lambda: t_stats(0), lambda: t_math(0, gp=True)],
                4: [
                    lambda: t_bc_aln(0), lambda: t_mlp(0),
                    lambda: t_stats(1), lambda: t_math(1, gp=True),
                ],
                5: [lambda: t_bc_aln(1), lambda: t_mlp(1)],
                6: [lambda: t_stats(2), lambda: t_math(2, gp=True)],
                7: [
                    lambda: t_mlp_mm(2),
                    lambda: t_stats(3),
                    lambda: t_math(3),
                    lambda: t_out(2),
                    lambda: t_bc_aln(3),
                    lambda: t_mlp_mm(3),
                    lambda: t_out(3),
                ],
            }
            k_sched_p7 = {
                4: [lambda: t_bc_aln(2)],
            }

            # ---------------- main contraction ----------------
            for p in range(NPAIR):
                kbt = kb_tiles[p]
                ps = pmain.tile([128, RB, D], F32, name="ps", tag="ps")
                for k in range(NK):
                    # the pair's two j-blocks run concurrently in the two
                    # column halves of the PE array (col groups from the
                    # psum base partition; x duplicated into both halves)
                    nc.tensor.matmul(
                        ps[0:64],
                        lhsT=xc_sb[:, k, 0:64],
                        rhs=kbt[:, k, 0, :, :],
                        start=(k == 0),
                        stop=(k == NK - 1),
                    )
                    nc.tensor.matmul(
                        ps[64:128],
                        lhsT=xc_sb[:, k, 64:128],
                        rhs=kbt[:, k, 1, :, :],
                        start=(k == 0),
                        stop=(k == NK - 1),
                    )
                    if p == NPAIR - 1:
                        for fn in k_sched_p7.get(k, ()):
                            fn()
                # d-reduction: DVE multiplies PSUM directly by the W^T
                # broadcast and reduces over d into the bf16 aT; the last
                # pair is split into row halves to shorten the endgame
                halves = (
                    [(0, RB // 2), (RB // 2, RB)] if p == NPAIR - 1 else [(0, RB)]
                )
                for r0, r1 in halves:
                    nr = r1 - r0
                    mw = mw_pool.tile([128, nr, D], BF16, name=f"mw{nr}", tag=f"mw{nr}")
                    nc.vector.tensor_mul(
                        mw.rearrange("p a b -> p (a b)"),
                        ps[:, r0:r1, :].rearrange("p a b -> p (a b)"),
                        wb_sb[:, r0 * D : r1 * D],
                    )
                    with nc.allow_low_precision(
                        reason="bf16 aT validated: fro rel err 1.55e-2 vs 2e-2 gate"
                    ):
                        nc.vector.tensor_reduce(
                            out=aT[:, RB * p + r0 : RB * p + r1],
                            in_=mw,
                            axis=mybir.AxisListType.X,
                            op=mybir.AluOpType.add,
                        )
                t_stacked(p)
                for fn in sched.get(p, ()):
                    fn()


    if split_waits:
        _split_matmul_waits(nc)
    return nc


def _split_matmul_waits(nc):
    """This walrus build rejects engine instructions carrying more than one
    semaphore wait ("Too many sync wait commands"). Peel all but the last
    wait off onto same-engine NoOps inserted immediately before the
    instruction — NoOps execute in queue order on the same sequencer, so the
    wait semantics are unchanged."""
    f = nc.m.functions[0]
    nop_id = 0
    for blk in f.blocks:
        insts = list(blk.instructions)
        out = []
        changed = False
        for inst in insts:
            si = inst.sync_info
            if (
                si is not None
                and si.on_wait is not None
                and len(si.on_wait) > 1
                and getattr(inst, "engine", None) is not None
            ):
                waits = list(si.on_wait)
                for w in waits[:-1]:
                    nop = mybir.InstNoOp(
                        name=f"I-mmwait-{nop_id}",
                        engine=inst.engine,
                        ins=[],
                        outs=[],
                        sync_info=mybir.SyncInfo(on_wait=[w], on_update=[]),
                    )
                    nop_id += 1
                    out.append(nop)
                inst.sync_info = mybir.SyncInfo(
                    on_wait=[waits[-1]], on_update=list(si.on_update or [])
                )
                changed = True
            out.append(inst)
        if changed:
            blk.instructions = out


def _get_nc():
    global _NC_CACHE
    if _NC_CACHE is None:
        _NC_CACHE = _build_nc()
    return _NC_CACHE


def _prep_shared(kernel_W, conv_bias, ln_scale, ln_bias, W1, b1, W2, b2):
    import ml_dtypes

    WT = kernel_W.T.astype(np.float32)  # [H, D]
    wb = np.broadcast_to(WT[:, None, :], (H, RB, D)).reshape(H, RB * D)
    # fold LayerNorm affine into the first MLP layer: the kernel computes
    # z = (a - mu) * inv_std, and  (z*s + b) @ W1 + b1 = z @ (s[:,None]*W1)
    # + (b1 + b @ W1)
    W1f = ln_scale[:, None].astype(np.float32) * W1.astype(np.float32)
    b1f = b1.astype(np.float32) + ln_bias.astype(np.float32) @ W1.astype(np.float32)

    bpak = np.zeros((128, BPAK_W), np.float32)
    bpak[0:64, OFF_WB : OFF_WB + 512] = wb
    bpak[64:128, OFF_WB : OFF_WB + 512] = wb
    bpak[0:64, OFF_W1Z : OFF_W1Z + FH] = W1f
    bpak[64:128, OFF_W1Z + FH : OFF_W1Z + 2 * FH] = W1f
    bpak[:, OFF_W2 : OFF_W2 + 128] = (
        W2.reshape(2, 128, H).transpose(1, 0, 2).reshape(128, 128)
    )
    bpak[0:64, OFF_SELS] = 1.0 / H
    bpak[64:128, OFF_SELS + 1] = 1.0 / H

    fpak = np.zeros((128, FPAK_W), np.float32)
    fpak[:, FOFF_CB] = np.tile(conv_bias, 2)
    fpak[:, FOFF_B1 : FOFF_B1 + 2] = b1f.reshape(2, 128).T
    fpak[0:64, FOFF_B2 : FOFF_B2 + 64] = np.broadcast_to(b2, (64, H))

    return dict(bpak=np.ascontiguousarray(bpak.astype(ml_dtypes.bfloat16))), fpak


def _prep_core_scale(xb, kernel_W, fpak, conv_bias):
    """Per-core NR constants: v0 = typical LN row variance estimate. The
    seed scale s = bf16(1/sqrt(v0)) rides the two ACT Square scales (so
    var arrives pre-divided by v0 and the NR runs from seed 1.0 with
    NEFF-constant immediates) and is restored via the sel2 broadcast."""
    import ml_dtypes

    WT = kernel_W.T.astype(np.float32)
    v0 = float(np.mean((xb.astype(np.float32) ** 2).sum(0) * (WT**2).sum(1)))
    s = np.float32(
        np.float32(1.0 / np.sqrt(v0)).astype(ml_dtypes.bfloat16).astype(np.float32)
    )
    sel2 = np.zeros((2, 128), np.float32)
    sel2[0, 0:64] = s
    sel2[1, 64:128] = s
    sv2 = np.full((2, 1), s, np.float32)
    fpc = fpak.copy()
    fpc[:, FOFF_SC] = s
    fpc[:, FOFF_CBS] = np.tile(conv_bias, 2) * s
    return (
        np.ascontiguousarray(sel2.astype(ml_dtypes.bfloat16)),
        np.ascontiguousarray(sv2),
        np.ascontiguousarray(fpc),
    )


def _prep_x(xb):
    import ml_dtypes

    # (N, H) -> (128, k, 2*H) bf16, with s = 128*k + p and x duplicated into
    # both column halves of the stationary operand
    xh = xb.astype(ml_dtypes.bfloat16)
    base = xh.reshape(NK, 128, H).transpose(1, 0, 2)
    return np.ascontiguousarray(np.concatenate([base, base], axis=2))


def _prep_kb_shard(shard):
    import ml_dtypes

    # shard (256, 1024, 32) f32 -> [pair, s%128, k, half, r, d] fp8 e3m4
    q = shard.reshape(NPAIR, 2, RB, NK, 128, D).transpose(0, 4, 3, 1, 2, 5)
    return np.ascontiguousarray(q.astype(ml_dtypes.float8_e3m4))


def kernel(
    x,
    kernel_basis,
    kernel_W,
    conv_bias,
    ln_scale,
    ln_bias,
    W1,
    b1,
    W2,
    b2,
):
    global LAST_EXEC_NS
    x = np.ascontiguousarray(np.asarray(x, np.float32))
    kb = np.ascontiguousarray(np.asarray(kernel_basis, np.float32))
    kernel_W = np.asarray(kernel_W, np.float32)
    conv_bias = np.asarray(conv_bias, np.float32)
    shared, fpak0 = _prep_shared(
        kernel_W,
        conv_bias,
        np.asarray(ln_scale, np.float32),
        np.asarray(ln_bias, np.float32),
        np.asarray(W1, np.float32),
        np.asarray(b1, np.float32),
        np.asarray(W2, np.float32),
        np.asarray(b2, np.float32),
    )
    xps = [_prep_x(x[b]) for b in range(B)]
    scs = [_prep_core_scale(x[b], kernel_W, fpak0, conv_bias) for b in range(B)]

    kbf = kb.reshape(B * N, N, D)
    in_maps = []
    for c in range(NCORES):
        b = c // (NCORES // B)
        hi = _prep_kb_shard(kbf[c * ROWS_PER_CORE : (c + 1) * ROWS_PER_CORE])
        in_maps.append(
            dict(
                kbh=hi, xcp2=xps[b], sel2=scs[b][0], sv2=scs[b][1],
                fpak=scs[b][2], **shared,
            )
        )

    nc = _get_nc()
    trace = bool(os.environ.get("KERNEL_BASS_TRACE"))
    res = run_bass_kernel_spmd(nc, in_maps, core_ids=list(range(NCORES)), trace=trace)
    LAST_EXEC_NS = res.exec_time_ns

    outs = np.concatenate([res.results[c]["out"] for c in range(NCORES)], axis=0)
    return outs.reshape(B, N, H)
